# revision 7
# baseline (speedup 1.0000x reference)
"""Trainium2 Bass kernel for nn_LossUnsupervisedAngle.

Math (per reference):
    xn = x / ||x||_2  (rows)
    mn = m / ||m||_2  (rows)
    y  = xn @ mn.T                       # [N, K] cosine sims, |y| <= 1
    p  = softmax(y, -1)
    ent_r = -sum_k p log p = ln(Z_r) - W_r / Z_r
        with Z_r = sum_k e^{y_rk},  W_r = sum_k y_rk e^{y_rk}
    out = mean_r(ent_r)

Because |y| <= 1, exp() needs no max-subtraction (e^y in [e^-1, e]), so the
entire softmax-entropy reduces to two fused streaming reductions per row.

Sharding: data-parallel on 8 cores, 8192 rows of x per core; m replicated.
The host pre-transposes each x shard (and casts to bf16) purely as a layout
choice so the contraction dim lands on SBUF partitions; all numerical work
(norms, normalize, matmul, softmax entropy, mean) happens on device. The
final per-core scalar sums are combined on host (equivalent of the
all-reduce of partial sums).

Per 128-row tile on device:
  DVE : v = sum_f x^2                  (tensor_tensor_reduce, fused)
  ACT : s = rsqrt(v) = exp(-0.5 ln v)  (Ln+Exp batched per 8 tiles; same
        activation-table set as the main Exp -> one table load total)
  PE  : Y[128,1024] = x_tile @ mn.T    (bf16, fp32 PSUM accumulate)
  ACT : E = exp(s*Y), Z = sum_k E      (single activation w/ vector scale +
        accum_out)
  DVE : W = sum_k (Y*s)*E              (scalar_tensor_tensor w/ accum_out)
Endgame (batched over the 64 tile-columns):
  ent = ln(Z) - W / Z ; row-sum on DVE; partition-sum on GPSIMD; DMA scalar.
"""

import os
import sys
from contextlib import ExitStack

import numpy as np

if "/opt/trn_rl_repo" not in sys.path:
    sys.path.insert(0, "/opt/trn_rl_repo")

import ml_dtypes

import concourse.bass as bass
import concourse.tile as tile
from concourse import bacc, mybir
from concourse import bass_isa
from concourse.bass_utils import run_bass_kernel_spmd
from concourse.masks import make_identity

dt = mybir.dt
AF = mybir.ActivationFunctionType
ALU = mybir.AluOpType

N_CORES = 8
N_TOTAL = 65536
F = 512  # feature dim
K = 1024  # num clusters
P = 128  # partitions
FC = F // P  # 4 f-chunks (contraction subtiles)
N_SHARD = N_TOTAL // N_CORES  # 8192 rows per core
GROUP = 8  # tiles per rsqrt batch


def build_kernel(n_shard=N_SHARD, group=GROUP):
    tiles = n_shard // P
    n_groups = tiles // group
    assert n_groups * group == tiles

    nc = bacc.Bacc("TRN2", target_bir_lowering=False, debug=False)

    xt_d = nc.dram_tensor("xt", [F, n_shard], dt.bfloat16, kind="ExternalInput")
    xn_d = nc.dram_tensor("xn", [n_shard, F], dt.bfloat16, kind="ExternalInput")
    m_d = nc.dram_tensor("m", [K, F], dt.float32, kind="ExternalInput")
    out_d = nc.dram_tensor("out", [1, 1], dt.float32, kind="ExternalOutput")

    MT = K // P  # 8 m row-tiles

    with tile.TileContext(nc) as tc, ExitStack() as ctx:
        const_pool = ctx.enter_context(tc.tile_pool(name="const", bufs=1))
        mload = ctx.enter_context(tc.tile_pool(name="mload", bufs=MT))
        mnorm = ctx.enter_context(tc.tile_pool(name="mnorm", bufs=MT))
        mnt_pool = ctx.enter_context(tc.tile_pool(name="mnt", bufs=1))
        stat = ctx.enter_context(tc.tile_pool(name="stat", bufs=1))
        lng = ctx.enter_context(tc.tile_pool(name="lng", bufs=2))
        xtp = ctx.enter_context(tc.tile_pool(name="xtp", bufs=3))
        xnp = ctx.enter_context(tc.tile_pool(name="xnp", bufs=3))
        ep = ctx.enter_context(tc.tile_pool(name="ep", bufs=2))
        scr = ctx.enter_context(tc.tile_pool(name="scr", bufs=2))
        psum_y = ctx.enter_context(
            tc.tile_pool(name="psum_y", bufs=2, space=bass.MemorySpace.PSUM)
        )
        psum_t = ctx.enter_context(
            tc.tile_pool(name="psum_t", bufs=2, space=bass.MemorySpace.PSUM)
        )

        ident = const_pool.tile([P, P], dt.float32)
        make_identity(nc, ident[:])

        # ---------------- m preprocessing (one-time) ----------------
        # norms of the 8 m row-tiles -> sm = rsqrt(v); mn = m*sm cast bf16;
        # PE-transpose mn into mnt[c][f, k] (f32 transpose, cast on copy-out).
        vm = stat.tile([P, MT], dt.float32)
        m_tiles = []
        for i in range(MT):
            mt = mload.tile([P, F], dt.float32, tag="mt")
            nc.sync.dma_start(mt[:], m_d[i * P : (i + 1) * P, :])
            m_tiles.append(mt)
            msq = scr.tile([P, F], dt.float32, tag="msq")
            nc.vector.scalar_tensor_tensor(
                out=msq[:],
                in0=mt[:],
                scalar=1.0,
                in1=mt[:],
                op0=ALU.mult,
                op1=ALU.mult,
                accum_out=vm[:, i : i + 1],
            )
        lnvm = stat.tile([P, MT], dt.float32)
        smv = stat.tile([P, MT], dt.float32)
        nc.scalar.activation(lnvm[:], vm[:], AF.Ln)
        nc.scalar.activation(smv[:], lnvm[:], AF.Exp, scale=-0.5)

        mn_tiles = []
        for i in range(MT):
            mnb = mnorm.tile([P, F], dt.float32, tag="mnb")
            nc.vector.tensor_scalar(
                out=mnb[:],
                in0=m_tiles[i][:],
                scalar1=smv[:, i : i + 1],
                scalar2=None,
                op0=ALU.mult,
            )
            mn_tiles.append(mnb)

        # mnt[c] holds mn.T chunk [128 (f), 1024 (k)] in bf16
        mnt = [
            mnt_pool.tile([P, K], dt.bfloat16, tag=f"mnt{c}", name=f"mnt{c}")
            for c in range(FC)
        ]
        for i in range(MT):
            for c in range(FC):
                pt = psum_t.tile([P, P], dt.float32, tag="pt")
                nc.tensor.transpose(
                    pt[:], mn_tiles[i][:, c * P : (c + 1) * P], ident[:]
                )
                nc.scalar.copy(mnt[c][:, i * P : (i + 1) * P], pt[:])

        # ---------------- main loop ----------------
        zbuf = stat.tile([P, tiles], dt.float32)
        wbuf = stat.tile([P, tiles], dt.float32)
        sbuf = stat.tile([P, tiles], dt.float32)
        vbuf = stat.tile([P, tiles], dt.float32)

        xt_r = xt_d.rearrange("(c p) n -> p c n", p=P)  # [128, 4, n_shard]

        for g in range(n_groups):
            for jj in range(group):
                j = g * group + jj
                xnt = xnp.tile([P, F], dt.bfloat16, tag="xnt")
                nc.sync.dma_start(xnt[:], xn_d[j * P : (j + 1) * P, :])
                xsq = scr.tile([P, F], dt.float32, tag="xsq")
                nc.vector.scalar_tensor_tensor(
                    out=xsq[:],
                    in0=xnt[:],
                    scalar=1.0,
                    in1=xnt[:],
                    op0=ALU.mult,
                    op1=ALU.mult,
                    accum_out=vbuf[:, j : j + 1],
                )
            lnv = lng.tile([P, group], dt.float32, tag="lnv")
            gs = slice(g * group, (g + 1) * group)
            nc.scalar.activation(lnv[:], vbuf[:, gs], AF.Ln)
            nc.scalar.activation(sbuf[:, gs], lnv[:], AF.Exp, scale=-0.5)

            for jj in range(group):
                j = g * group + jj
                xtt = xtp.tile([P, FC, P], dt.bfloat16, tag="xtt")
                nc.sync.dma_start(xtt[:], xt_r[:, :, j * P : (j + 1) * P])

                ypsum = psum_y.tile([P, K], dt.float32, tag="y")
                for h in range(K // 512):
                    for c in range(FC):
                        nc.tensor.matmul(
                            ypsum[:, h * 512 : (h + 1) * 512],
                            xtt[:, c, :],
                            mnt[c][:, h * 512 : (h + 1) * 512],
                            start=(c == 0),
                            stop=(c == FC - 1),
                        )

                e_t = ep.tile([P, K], dt.float32, tag="e")
                nc.scalar.activation(
                    e_t[:],
                    ypsum[:],
                    AF.Exp,
                    scale=sbuf[:, j : j + 1],
                    accum_out=zbuf[:, j : j + 1],
                )
                wscr = scr.tile([P, K], dt.float32, tag="wscr")
                nc.vector.scalar_tensor_tensor(
                    out=wscr[:],
                    in0=ypsum[:],
                    scalar=sbuf[:, j : j + 1],
                    in1=e_t[:],
                    op0=ALU.mult,
                    op1=ALU.mult,
                    accum_out=wbuf[:, j : j + 1],
                )

        # ---------------- endgame ----------------
        lnz = stat.tile([P, tiles], dt.float32)
        nc.scalar.activation(lnz[:], zbuf[:], AF.Ln)
        rz = stat.tile([P, tiles], dt.float32)
        nc.vector.reciprocal(rz[:], zbuf[:])
        t1 = stat.tile([P, tiles], dt.float32)
        nc.vector.tensor_mul(t1[:], wbuf[:], rz[:])
        ent = stat.tile([P, tiles], dt.float32)
        nc.vector.tensor_sub(ent[:], lnz[:], t1[:])
        entp = stat.tile([P, 1], dt.float32)
        nc.vector.tensor_reduce(entp[:], ent[:], axis=mybir.AxisListType.X, op=ALU.add)
        entall = stat.tile([P, 1], dt.float32)
        nc.gpsimd.partition_all_reduce(
            entall[:], entp[:], channels=P, reduce_op=bass_isa.ReduceOp.add
        )
        nc.sync.dma_start(out_d[:, :], entall[0:1, :])

    nc.compile()
    return nc


_NC_CACHE = {}


def _get_nc():
    if "nc" not in _NC_CACHE:
        _NC_CACHE["nc"] = build_kernel()
    return _NC_CACHE["nc"]


def _run(x, m, **spmd_kwargs):
    x = np.asarray(x, dtype=np.float32)
    m = np.asarray(m, dtype=np.float32)
    assert x.shape == (N_TOTAL, F) and m.shape == (K, F)

    nc = _get_nc()
    xb = x.astype(ml_dtypes.bfloat16)
    in_maps = []
    for c in range(N_CORES):
        xs = xb[c * N_SHARD : (c + 1) * N_SHARD]
        in_maps.append(
            {
                "xt": np.ascontiguousarray(xs.T),
                "xn": np.ascontiguousarray(xs),
                "m": m,
            }
        )
    res = run_bass_kernel_spmd(nc, in_maps, list(range(N_CORES)), **spmd_kwargs)
    total = sum(float(r["out"][0, 0]) for r in res.results) / float(N_TOTAL)
    t = np.float32(total)
    return (t, t, np.float32(0.0)), res


def kernel(x, m):
    out, _ = _run(x, m)
    return out


if __name__ == "__main__":
    # quick smoke: random input
    rng = np.random.default_rng(0)
    x = rng.standard_normal((N_TOTAL, F), dtype=np.float32)
    m = rng.standard_normal((K, F), dtype=np.float32)
    print(kernel(x, m))
